# revision 37
# baseline (speedup 1.0000x reference)
"""GAT 2-layer node classifier on 8 Trainium2 NeuronCores.

Strategy (1D dst-node partitioning + chunked src gather):
  - dst nodes sharded contiguously across 8 cores (12500 each, natural order)
  - node phase computes per-node rows [feat64|el8|er8|pad] fp16 (256B) which
    are AllGathered into a 100360-row table (12545 rows per core incl. one
    dummy row whose el is -3e4 so padded slots contribute exp() = 0).
  - the per-edge gather uses dma_gather (Ant Q7 kernel, int16 indices):
    the table is addressed through 4 windows of 2 source cores each
    (25090 rows < 32767).  Per window ("chunk") each core re-sorts its dst
    nodes by within-chunk degree and packs 128 dst per group with in-edge
    slots padded to the group's max chunk-degree, giving a near-tight grid
    (~2.4% padding).  Per chunk the partial softmax sums [U64|Z8] are
    written to a DRAM table in chunk order; a renorm phase gathers the 4
    partials back into natural order (gather-based realignment), divides,
    applies bias/relu and runs the layer-2 projection.  Layer 2 repeats the
    edge phase with the same grids/indices on the layer-2 table.
"""

import sys
import types

import numpy as np

# ---------------------------------------------------------------------------
# environment shims (self-contained: only touches in-process state)
# ---------------------------------------------------------------------------


def _ensure_axon_hooks():
    """concourse.bass_utils imports antenv.axon_hooks when tracing under
    axon; some images lack the module. Provide an in-process shim and
    register the real ctypes NTFF hook so trace=True yields exec times."""
    try:
        import antenv.axon_hooks  # noqa: F401
        return
    except Exception:
        pass
    try:
        import antenv
    except Exception:
        return
    mod = types.ModuleType("antenv.axon_hooks")
    mod._hook = None

    def set_axon_ntff_profile_hook(hook):
        mod._hook = hook

    def get_axon_ntff_profile_hook():
        return mod._hook

    mod.set_axon_ntff_profile_hook = set_axon_ntff_profile_hook
    mod.get_axon_ntff_profile_hook = get_axon_ntff_profile_hook
    sys.modules["antenv.axon_hooks"] = mod
    antenv.axon_hooks = mod
    try:
        from trn_agent_boot.trn_boot import _ntff_profile_via_ctypes
        hook = _ntff_profile_via_ctypes("/opt/axon/libaxon_pjrt.so")
        if hook is not None:
            mod._hook = hook
    except Exception:
        pass


_ensure_axon_hooks()

import concourse.bass as bass          # noqa: E402
import concourse.mybir as mybir        # noqa: E402
import concourse.tile as tile          # noqa: E402
from concourse import library_config   # noqa: E402
from concourse.vector_clock import ScopedClock  # noqa: E402
from concourse.bass_utils import run_bass_kernel_spmd  # noqa: E402

F32 = mybir.dt.float32
F16 = mybir.dt.float16
BF16 = mybir.dt.bfloat16
I16 = mybir.dt.int16
AF = mybir.ActivationFunctionType
OP = mybir.AluOpType
AX = mybir.AxisListType


def _patched_drain_and_barrier(self, tick_clock, wait_clock):
    # this walrus build rejects multi-wait instructions; emit one wait per
    # nop before the tail drain instead of stacking them on the drain.
    nc = self.nc
    probe = nc.sync.nop(nofuse=True)
    wait_clock.add_sem_waits(probe.ins, ScopedClock({None: tick_clock.global_clock}))
    waits = list(probe.ins.sync_info.on_wait or []) if probe.ins.sync_info else []
    if waits:
        probe.ins.sync_info = mybir.SyncInfo(on_wait=[waits[0]], on_update=[])
        for w in waits[1:]:
            nop = nc.sync.nop(nofuse=True)
            nop.ins.sync_info = mybir.SyncInfo(on_wait=[w], on_update=[])
    nc.sync.drain()
    nc.all_engine_barrier()
    popped = nc._tile_sem_poison_stack.pop()
    assert popped is self._sem_poison
    nc.clear_and_free_semaphores(list(self.sems.allocated().values()))
    nc.all_engine_barrier()


tile.TileContext._drain_and_barrier = _patched_drain_and_barrier


def _split_waits(nc, max_waits=1):
    """Post-pass: any instruction carrying more than max_waits sem-waits gets
    preceding same-engine NoOps carrying the excess."""
    uid = [0]
    for f in nc.m.functions:
        for bb in f.blocks:
            new_insts = []
            for inst in bb.instructions:
                si = getattr(inst, "sync_info", None)
                if si is not None and si.on_wait and len(si.on_wait) > max_waits:
                    waits = list(si.on_wait)
                    excess, keep = waits[:-max_waits], waits[-max_waits:]
                    for i in range(0, len(excess), max_waits):
                        uid[0] += 1
                        new_insts.append(mybir.InstNoOp(
                            name=f"waitsplit-{uid[0]}-{inst.name}",
                            sync_info=mybir.SyncInfo(
                                on_wait=excess[i:i + max_waits], on_update=[]),
                            bass_nofuse=True,
                            engine=inst.engine,
                        ))
                    inst.sync_info = mybir.SyncInfo(
                        on_wait=keep, on_update=list(si.on_update or []))
                new_insts.append(inst)
            bb.instructions = new_insts


# ---------------------------------------------------------------------------
# problem constants (hardcoded per spec)
# ---------------------------------------------------------------------------
N_CORES = 8
V = 100000            # nodes
S = V // N_CORES      # nodes per core shard (12500)
F_IN = 256
H1, D1 = 8, 8         # layer-1 heads x dim
HD1 = H1 * D1         # 64
C2 = 40               # classes (layer-2 single head)
NEG_SLOPE = 0.2
EL_NEG = -30000.0     # dummy-row attention logit (exp -> 0)
G = (S + 127) // 128  # 98 groups of 128 dst nodes
SP = G * 128          # 12544 padded shard size
W = SP + 1            # 12545 table rows per core (last = dummy)
NCH = 4               # src chunks (2 cores per window)
WIN = 2 * W           # 25090 rows per chunk window
ROWB = 128            # fp16 elements per table row (256B)
CB = 64               # max grid columns per gather batch (SBUF window)
SC = 8                # grid columns per dma_gather sub-call (<=1024 idxs)
RB = 8                # renorm groups per realign gather call (<=1024 idxs)
MAXL = 32             # per-(group,chunk) slot-column bound

# module-level knobs (test harness pokes these)
PROFILE = False
DEBUG = False
PHASES = 6   # debug: 1=node 2=+AG1 3=+edgeL1 4=+renorm1 5=+AG2+edgeL2 6=full
EDGE_SUB = 4  # debug: 1=ER only 2=+F gathers 3=+scores 4=full
SKIP_AG = False  # debug: skip collectives
LAST_EXEC_NS = None
LAST_RESULTS = None


# ---------------------------------------------------------------------------
# host-side graph preprocessing (integer work only)
# ---------------------------------------------------------------------------

def _wrap16(flat):
    """int array [n] (n % 16 == 0) -> dma_gather idx layout [128, n//16]:
    idx i at partition i%16, col i//16, replicated to 128 partitions."""
    a = np.asarray(flat, np.int16).reshape(-1, 16).T    # [16, n//16]
    return np.tile(a, (8, 1)).copy()                    # [128, n//16]


def _host_prep(src, dst):
    src = np.asarray(src).astype(np.int64)
    dst = np.asarray(dst).astype(np.int64)
    order = np.argsort(dst, kind="stable")
    src_s = src[order]
    dst_s = dst[order]
    bounds = np.searchsorted(dst_s, np.arange(N_CORES + 1) * S)

    # table row of a node: core cc = v // S, row = cc*W + (v - cc*S)
    cc_all = src_s // S
    trow_all = cc_all * W + (src_s - cc_all * S)        # global table row
    chunk_all = cc_all // 2                             # 0..3
    lrow_all = trow_all - chunk_all * WIN               # window-local row

    per_core = []
    Lkg_shared = np.zeros((NCH, G), np.int64)
    for c in range(N_CORES):
        sl = slice(bounds[c], bounds[c + 1])
        ld = dst_s[sl] - c * S
        ck = chunk_all[sl]
        lr = lrow_all[sl]
        chunks = []
        for k in range(NCH):
            m = ck == k
            ldk, lrk = ld[m], lr[m]
            degk = np.bincount(ldk, minlength=SP).astype(np.int64)  # ext ids
            pk = np.argsort(-degk, kind="stable").astype(np.int64)
            inv = np.empty(SP, np.int64)
            inv[pk] = np.arange(SP)
            Lg = degk[pk[np.arange(G) * 128]]
            Lkg_shared[k] = np.maximum(Lkg_shared[k], Lg)
            # edge list sorted by (chunk-rank of dst, stable)
            o2 = np.argsort(inv[ldk], kind="stable")
            chunks.append(dict(degk=degk, pk=pk, inv=inv,
                               e_lrow=lrk[o2], e_pos=inv[ldk][o2]))
        per_core.append(chunks)

    # shared grid: per chunk, columns for groups with L>0
    grids = []          # per chunk: list of (g, L, colstart)
    cols_k = []
    for k in range(NCH):
        col = 0
        gl = []
        for g in range(G):
            L = int(Lkg_shared[k][g])
            if L == 0:
                continue
            gl.append((g, L, col))
            col += L
        grids.append(gl)
        cols_k.append(col)

    # per-core slot/realign index arrays
    cores_data = []
    for c in range(N_CORES):
        idx_e, idx_er, idx_uz = [], [], []
        for k in range(NCH):
            ch = per_core[c][k]
            dummy = 2 * (k * W) + W - 1 - k * WIN       # first core's dummy row
            dummy = W - 1                                # window-local: core 2k dummy
            ncols = cols_k[k]
            slots = np.full((ncols, 128), dummy, np.int64)
            degk, pk = ch["degk"], ch["pk"]
            starts = np.zeros(SP, np.int64)
            np.cumsum(degk[pk][:-1], out=starts[1:])     # start in e_lrow per rank
            for (g, L, col) in grids[k]:
                ranks = np.arange(g * 128, (g + 1) * 128)
                dg = degk[pk[ranks]]
                st = starts[ranks]
                ar = np.arange(L)
                mask = ar[None, :] < dg[:, None]
                pos = np.minimum(st[:, None] + ar[None, :],
                                 max(len(ch["e_lrow"]) - 1, 0))
                vals = (ch["e_lrow"][pos] if len(ch["e_lrow"])
                        else np.zeros_like(pos))
                gs = np.where(mask, vals, dummy)         # [128, L]
                slots[col:col + L, :] = gs.T
            assert slots.max() < WIN
            idx_e.append(_wrap16(slots.reshape(-1)))
            # er realign: grid position i -> local shard-table row pk[i]
            idx_er.append(_wrap16(ch["pk"]))
            # uz realign: natural position i -> chunk rank inv[i]
            idx_uz.append(_wrap16(ch["inv"]))
        cores_data.append(dict(idx_e=idx_e, idx_er=idx_er, idx_uz=idx_uz))

    # gather call batches per chunk: runs of groups with sum(L) <= CB
    batches = []        # per chunk: list of (colstart, ncols, [(g,L,off)...])
    for k in range(NCH):
        bl = []
        cur = []
        cur_cols = 0
        cur_start = 0
        for (g, L, col) in grids[k]:
            if cur_cols + L > CB:
                bl.append((cur_start, cur_cols, cur))
                cur, cur_cols, cur_start = [], 0, col
            cur.append((g, L, cur_cols))
            cur_cols += L
        if cur:
            bl.append((cur_start, cur_cols, cur))
        batches.append(bl)

    return cores_data, grids, cols_k, batches, Lkg_shared


# ---------------------------------------------------------------------------
# device program
# ---------------------------------------------------------------------------

def _build_program(cols_k, batches, grids):
    nc = bass.Bass("TRN2", target_bir_lowering=False, debug=False,
                   num_devices=N_CORES, num_swdge_queues=4)

    def dram_in(name, shape, dt=F32):
        return nc.dram_tensor(name, list(shape), dt, kind="ExternalInput").ap()

    xT = dram_in("xT", [F_IN, SP])
    W1e_d = dram_in("W1e", [F_IN, 80])
    W2e_d = dram_in("W2e", [HD1, 42])
    b1_d = dram_in("b1", [128, HD1])
    b2_d = dram_in("b2", [128, C2])
    ident_d = dram_in("ident", [128, 128])
    drow1_d = dram_in("drow1", [1, ROWB], F16)
    drow2_d = dram_in("drow2", [1, ROWB], F16)
    idxe_d = [dram_in(f"idxe{k}", [128, cols_k[k] * 8], I16) for k in range(NCH)]
    idxer_d = [dram_in(f"idxer{k}", [128, SP // 16], I16) for k in range(NCH)]
    idxuz_d = [dram_in(f"idxuz{k}", [128, SP // 16], I16) for k in range(NCH)]

    out_shard = nc.dram_tensor("out_shard", [SP, C2], F32,
                               kind="ExternalOutput").ap()

    with tile.TileContext(nc) as tc:
        with (
            tc.tile_pool(name="dram", bufs=1, space="DRAM") as dram,
            tc.tile_pool(name="const", bufs=1) as constp,
            tc.tile_pool(name="node", bufs=2) as nodep,
            tc.tile_pool(name="idxp", bufs=1) as idxp,
            tc.tile_pool(name="erp", bufs=1) as erp,
            tc.tile_pool(name="uzp", bufs=1) as uzp,
            tc.tile_pool(name="gath", bufs=3) as gath,
            tc.tile_pool(name="work", bufs=2) as work,
            tc.tile_pool(name="rz", bufs=2) as rzp,
            tc.tile_pool(name="psum", bufs=2, space="PSUM") as psum,
        ):
            # Pool has only 48 allocatable registers; dma_gather burns one
            # per to_reg(num_idxs) call — cache by value.
            _regs = {}
            def nreg(v):
                if v not in _regs:
                    _regs[v] = nc.gpsimd.to_reg(v)
                return _regs[v]

            _q = [0]
            def qrr():
                _q[0] = (_q[0] + 1) % 4
                return _q[0]

            # ---- persistent DRAM tables -----------------------------------
            t1_shard = dram.tile([W, ROWB], F16)
            t1_full = dram.tile([N_CORES * W, ROWB], F16)
            t2_shard = dram.tile([W, ROWB], F16)
            t2_full = dram.tile([N_CORES * W, ROWB], F16)
            UZ1 = [dram.tile([SP, ROWB], F16, name=f"UZ1_{k}", tag=f"UZ1_{k}")
                   for k in range(NCH)]
            UZ2 = [dram.tile([SP, ROWB], F16, name=f"UZ2_{k}", tag=f"UZ2_{k}")
                   for k in range(NCH)]
            t1_full[:].tensor.mls.addr_space = "Shared"
            t2_full[:].tensor.mls.addr_space = "Shared"

            # ---- constants into SBUF --------------------------------------
            _cn = [0]
            def const_load(src_ap, shape, dt=F32):
                _cn[0] += 1
                t = constp.tile(shape, dt, tag=f"const{_cn[0]}")
                nc.sync.dma_start(out=t[:], in_=src_ap)
                return t

            W1a = const_load(W1e_d[0:128, :], [128, 80])
            W1b = const_load(W1e_d[128:256, :], [128, 80])
            W2sb = const_load(W2e_d[:, :], [HD1, 42])
            b1 = const_load(b1_d[:, :], [128, HD1])
            b2 = const_load(b2_d[:, :], [128, C2])
            ident = const_load(ident_d[:, :], [128, 128])

            # dummy rows of the gather tables
            nc.sync.dma_start(out=t1_shard[W - 1:W, :], in_=drow1_d[:, :])
            nc.sync.dma_start(out=t2_shard[W - 1:W, :], in_=drow2_d[:, :])

            # ---- node phase: [feat64|el8|er8] fp16 rows for own shard -----
            NB = 4
            for n in range(G):
                if n % NB == 0:
                    nw = min(NB, G - n) * 128
                    cs4 = slice(n * 128, n * 128 + nw)
                    xa = nodep.tile([128, NB * 128], F32, tag="xa")
                    xb = nodep.tile([128, NB * 128], F32, tag="xb")
                    nc.scalar.dma_start(out=xa[:, 0:nw], in_=xT[0:128, cs4])
                    nc.scalar.dma_start(out=xb[:, 0:nw], in_=xT[128:256, cs4])
                k = (n % NB) * 128
                p1 = psum.tile([128, 80], F32, tag="p1")
                nc.tensor.matmul(out=p1[:], lhsT=xa[:, k:k + 128], rhs=W1a[:],
                                 start=True, stop=False)
                nc.tensor.matmul(out=p1[:], lhsT=xb[:, k:k + 128], rhs=W1b[:],
                                 start=False, stop=True)
                if n % NB == 0:
                    S4 = nodep.tile([128, NB * ROWB], F16, tag="S4")
                j = n % NB
                nc.scalar.copy(out=S4[:, j * ROWB:j * ROWB + 80], in_=p1[:])
                if n % NB == NB - 1 or n == G - 1:
                    m = n % NB + 1
                    a = (n - m + 1) * 128
                    nc.sync.dma_start(
                        out=t1_shard[a:a + m * 128, :]
                            .rearrange("(j p) r -> p j r", p=128),
                        in_=S4[:, 0:m * ROWB]
                            .rearrange("p (j r) -> p j r", r=ROWB))

            # ---- AllGather layer-1 table ----------------------------------
            if PHASES >= 2 and not SKIP_AG:
                nc.gpsimd.collective_compute(
                    "AllGather", OP.bypass,
                    replica_groups=[list(range(N_CORES))],
                    ins=[t1_shard[0:W, :].opt()],
                    outs=[t1_full[:, :].opt()],
                )

            # ---- edge phase (shared between both layers) ------------------
            def edge_phase(tfull, tshard, UZ, FC, elc, erc):
                """FC: feature count (64/40); elc/erc: el/er col in row."""
                ctx = nc.allow_low_precision(
                    reason="per-chunk partials in fp16; <=24-term sums")
                ctx.__enter__()
                for k in range(NCH):
                    win = tfull[k * WIN:(k + 1) * WIN, :]
                    idxs = idxp.tile([128, cols_k[k] * 8], I16, tag="idxs")
                    nc.sync.dma_start(out=idxs[:], in_=idxe_d[k][:, :])
                    ER = erp.tile([128, G * ROWB], F16, tag="ER")
                    if EDGE_SUB == 0:
                        nc.vector.memset(ER[:], 0.0)
                    else:
                        EB = 8   # groups per call (1024-idx gather limit)
                        for g0 in range(0, G, EB):
                            ng = min(EB, G - g0)
                            nc.gpsimd.dma_gather(
                                ER[:, g0 * ROWB:(g0 + ng) * ROWB]
                                    .rearrange("p (b r) -> p b r", r=ROWB),
                                tshard[0:W, :],
                                ier[k][:, g0 * 8:(g0 + ng) * 8],
                                ng * 128, nreg(ng * 128), ROWB,
                                queue_num=qrr())
                    ERv = ER[:].rearrange("p (b r) -> p b r", r=ROWB)
                    UZSB = uzp.tile([128, G * ROWB], F16, tag="UZSB")
                    if EDGE_SUB < 4:
                        nc.vector.memset(UZSB[:], 0.0)
                    if EDGE_SUB <= 1:
                        nc.sync.dma_start(
                            out=UZ[k][0:SP, :]
                                .rearrange("(g p) r -> p g r", p=128),
                            in_=UZSB[:].rearrange("p (g r) -> p g r", r=ROWB))
                        continue
                    for (colstart, ncols, gl) in batches[k]:
                        F = gath.tile([128, CB * ROWB], F16, tag="F")
                        for c0 in range(0, ncols, SC):
                            nc2 = min(SC, ncols - c0)
                            nc.gpsimd.dma_gather(
                                F[:, c0 * ROWB:(c0 + nc2) * ROWB]
                                    .rearrange("p (b r) -> p b r", r=ROWB),
                                win,
                                idxs[:, (colstart + c0) * 8:
                                     (colstart + c0 + nc2) * 8],
                                nc2 * 128, nreg(nc2 * 128), ROWB,
                                queue_num=qrr())
                        if EDGE_SUB == 2:
                            continue
                        Fv = F[:, 0:ncols * ROWB].rearrange(
                            "p (b r) -> p b r", r=ROWB)
                        nh = NH[FC]
                        A = work.tile([128, CB * 8], F32, tag="A")
                        for (g, L, off) in gl:
                            nc.vector.tensor_add(
                                out=A[:, off * nh:(off + L) * nh].rearrange(
                                    "p (l h) -> p l h", h=nh),
                                in0=Fv[:, off:off + L, elc:elc + nh],
                                in1=ERv[:, g:g + 1, erc:erc + nh]
                                    .to_broadcast([128, L, nh]))
                        na = ncols * nh
                        LR = work.tile([128, CB * 8], F32, tag="LR")
                        nc.vector.tensor_scalar_mul(
                            out=LR[:, 0:na], in0=A[:, 0:na], scalar1=NEG_SLOPE)
                        nc.vector.tensor_tensor(
                            out=LR[:, 0:na], in0=A[:, 0:na], in1=LR[:, 0:na],
                            op=OP.max)
                        EX = work.tile([128, CB * 8], F16, tag="EX")
                        nc.scalar.activation(
                            out=EX[:, 0:na], in_=LR[:, 0:na], func=AF.Exp)
                        if EDGE_SUB == 3:
                            continue
                        for (g, L, off) in gl:
                            nc.vector.reduce_sum(
                                out=UZSB[:, g * ROWB + FC:g * ROWB + FC + nh],
                                in_=EX[:, off * nh:(off + L) * nh].rearrange(
                                    "p (l h) -> p h l", h=nh), axis=AX.X)
                            P = work.tile([128, MAXL * HD1], F16, tag="P")
                            nc.vector.tensor_mul(
                                out=P[:, 0:L * FC].rearrange(
                                    "p (l h j) -> p l h j", h=nh, j=FC // nh),
                                in0=Fv[:, off:off + L, 0:FC].rearrange(
                                    "p l (h j) -> p l h j", h=nh),
                                in1=EX[:, off * nh:(off + L) * nh].rearrange(
                                    "p (l h) -> p l h", h=nh)
                                    .rearrange("p l (h o) -> p l h o", o=1)
                                    .to_broadcast([128, L, nh, FC // nh]))
                            nc.vector.reduce_sum(
                                out=UZSB[:, g * ROWB:g * ROWB + FC],
                                in_=P[:, 0:L * FC].rearrange(
                                    "p (l f) -> p f l", f=FC), axis=AX.X)
                    # groups with no columns in this chunk: zero their U/Z
                    have = {g for (g, L, off) in
                            [t for (_, _, gl2) in batches[k] for t in gl2]}
                    for g in range(G):
                        if g not in have:
                            nc.vector.memset(
                                UZSB[:, g * ROWB:g * ROWB + FC + NH[FC]], 0.0)
                    nc.sync.dma_start(
                        out=UZ[k][0:SP, :].rearrange("(g p) r -> p g r", p=128),
                        in_=UZSB[:].rearrange("p (g r) -> p g r", r=ROWB))
                ctx.__exit__(None, None, None)

            NH = {HD1: H1, C2: 1}

            # ---- renorm + next-layer node phase ---------------------------
            def renorm_phase(UZ, idxuz, FC, emit_group):
                nh = NH[FC]
                nb = (G + RB - 1) // RB
                for b in range(nb):
                    g0 = b * RB
                    ng = min(RB, G - g0)
                    RZ = []
                    for k in range(NCH):
                        r = rzp.tile([128, RB * ROWB], F16, tag=f"RZ{k}")
                        nc.gpsimd.dma_gather(
                            r[:, 0:ng * ROWB]
                                .rearrange("p (b r) -> p b r", r=ROWB),
                            UZ[k][0:SP, :],
                            iuz[k][:, g0 * 8:(g0 + ng) * 8],
                            ng * 128, nreg(ng * 128), ROWB,
                            queue_num=qrr())
                        RZ.append(r)
                    nb2 = ng * ROWB
                    T0 = work.tile([128, RB * ROWB], F32, tag="T0")
                    nc.vector.tensor_add(out=T0[:, 0:nb2],
                                         in0=RZ[0][:, 0:nb2], in1=RZ[1][:, 0:nb2])
                    T1 = work.tile([128, RB * ROWB], F32, tag="T1")
                    nc.vector.tensor_add(out=T1[:, 0:nb2],
                                         in0=RZ[2][:, 0:nb2], in1=RZ[3][:, 0:nb2])
                    UZf = work.tile([128, RB * ROWB], F32, tag="UZf")
                    nc.vector.tensor_add(out=UZf[:, 0:nb2],
                                         in0=T0[:, 0:nb2], in1=T1[:, 0:nb2])
                    for gg in range(ng):
                        g = g0 + gg
                        c0 = gg * ROWB
                        rinv = work.tile([128, 8], F32, tag="rinv")
                        nc.vector.reciprocal(out=rinv[:, 0:nh],
                                             in_=UZf[:, c0 + FC:c0 + FC + nh])
                        O = work.tile([128, HD1], F32, tag="O")
                        nc.vector.tensor_mul(
                            out=O[:, 0:FC].rearrange("p (h j) -> p h j", h=nh),
                            in0=UZf[:, c0:c0 + FC].rearrange(
                                "p (h j) -> p h j", h=nh),
                            in1=rinv[:, 0:nh].rearrange("p (h o) -> p h o", o=1)
                                .to_broadcast([128, nh, FC // nh]))
                        emit_group(g, O)

            # layer-1 renorm group: h = relu(O + b1); project to layer-2 row
            S42 = [None]
            def emit_l1(g, O):
                Ht = work.tile([128, HD1], F32, tag="Ht")
                nc.vector.tensor_add(out=Ht[:], in0=O[:, 0:HD1], in1=b1[:])
                nc.scalar.activation(out=Ht[:], in_=Ht[:], func=AF.Relu)
                pT = psum.tile([HD1, 128], F32, tag="pT")
                nc.tensor.transpose(out=pT[:], in_=Ht[:], identity=ident[:])
                hT = work.tile([HD1, 128], F32, tag="hT")
                nc.scalar.copy(out=hT[:], in_=pT[:])
                p2 = psum.tile([128, 42], F32, tag="p2")
                nc.tensor.matmul(out=p2[:], lhsT=hT[:], rhs=W2sb[:],
                                 start=True, stop=True)
                if g % NB == 0:
                    S42[0] = nodep.tile([128, NB * ROWB], F16, name="S42",
                                        tag="S42")
                j = g % NB
                nc.scalar.copy(out=S42[0][:, j * ROWB:j * ROWB + 42], in_=p2[:])
                if g % NB == NB - 1 or g == G - 1:
                    m = g % NB + 1
                    a = (g - m + 1) * 128
                    nc.sync.dma_start(
                        out=t2_shard[a:a + m * 128, :]
                            .rearrange("(j p) r -> p j r", p=128),
                        in_=S42[0][:, 0:m * ROWB]
                            .rearrange("p (j r) -> p j r", r=ROWB))

            # layer-2 renorm group: out = O + b2
            O4 = [None]
            def emit_l2(g, O):
                if g % NB == 0:
                    O4[0] = nodep.tile([128, NB * C2], F32, name="O4", tag="O4")
                j = g % NB
                nc.vector.tensor_add(out=O4[0][:, j * C2:(j + 1) * C2],
                                     in0=O[:, 0:C2], in1=b2[:])
                if g % NB == NB - 1 or g == G - 1:
                    m = g % NB + 1
                    a = (g - m + 1) * 128
                    nc.sync.dma_start(
                        out=out_shard[a:a + m * 128, :]
                            .rearrange("(j p) r -> p j r", p=128),
                        in_=O4[0][:, 0:m * C2]
                            .rearrange("p (j r) -> p j r", r=C2))

            # ---- idx tiles for er/uz realign (persistent, small) ----------
            ier = [const_load(idxer_d[k][:, :], [128, SP // 16], I16)
                   for k in range(NCH)]
            iuz = [const_load(idxuz_d[k][:, :], [128, SP // 16], I16)
                   for k in range(NCH)]

            # ---- run the phases -------------------------------------------
            if PHASES >= 3:
                edge_phase(t1_full, t1_shard, UZ1, HD1, 64, 72)
            if PHASES >= 4:
                renorm_phase(UZ1, iuz, HD1, emit_l1)
            if PHASES >= 5:
                nc.gpsimd.collective_compute(
                    "AllGather", OP.bypass,
                    replica_groups=[list(range(N_CORES))],
                    ins=[t2_shard[0:W, :].opt()],
                    outs=[t2_full[:, :].opt()],
                )
                edge_phase(t2_full, t2_shard, UZ2, C2, 40, 41)
            if PHASES >= 6:
                renorm_phase(UZ2, iuz, C2, emit_l2)
            if PHASES < 6:
                zo = nodep.tile([128, C2], F32, tag="zo", name="zo")
                nc.vector.memset(zo[:], 0.0)
                nc.sync.dma_start(
                    out=out_shard[0:128, :], in_=zo[:])

    # Raw Bass skips Bacc's library/ISA lowering passes; without them the
    # NEFF compiler sees empty .instr on extended insts -> "ISA wrong length".
    import bass_rust as _bass_rust
    inst_type_to_lib_mask = {}
    for lib in library_config.all_libraries:
        for t in lib.instructions:
            inst_type_to_lib_mask[t] = (inst_type_to_lib_mask.get(t, 0)
                                        | (1 << lib.index))
    _bass_rust.insert_library_loads(
        nc, inst_type_to_lib_mask, len(library_config.all_libraries),
        library_config.standard.index)
    mybir.codegen_inst_isa_subclasses(nc)

    _split_waits(nc)
    return nc


# ---------------------------------------------------------------------------
# entry point
# ---------------------------------------------------------------------------

def kernel(x, W1, attn_l1, attn_r1, b1, W2, attn_l2, attn_r2, b2, src, dst):
    global LAST_EXEC_NS, LAST_RESULTS
    x = np.asarray(x, np.float32)
    cores_data, grids, cols_k, batches, Lkg = _host_prep(src, dst)
    nc = _build_program(cols_k, batches, grids)

    W1f = np.asarray(W1, np.float32)
    al1 = np.asarray(attn_l1, np.float32).reshape(H1, D1)
    ar1 = np.asarray(attn_r1, np.float32).reshape(H1, D1)
    Wl = (W1f.reshape(F_IN, H1, D1) * al1[None]).sum(-1)
    Wr = (W1f.reshape(F_IN, H1, D1) * ar1[None]).sum(-1)
    W1e = np.concatenate([W1f, Wl, Wr], axis=1).astype(np.float32)

    W2f = np.asarray(W2, np.float32)
    al2 = np.asarray(attn_l2, np.float32).reshape(1, C2)
    ar2 = np.asarray(attn_r2, np.float32).reshape(1, C2)
    Wl2 = (W2f * al2).sum(-1, keepdims=True)
    Wr2 = (W2f * ar2).sum(-1, keepdims=True)
    W2e = np.concatenate([W2f, Wl2, Wr2], axis=1).astype(np.float32)

    drow1 = np.zeros((1, ROWB), np.float16)
    drow1[0, 64:72] = EL_NEG
    drow2 = np.zeros((1, ROWB), np.float16)
    drow2[0, 40] = EL_NEG

    common = {
        "W1e": W1e,
        "W2e": W2e,
        "b1": np.tile(np.asarray(b1, np.float32).reshape(1, HD1), (128, 1)),
        "b2": np.tile(np.asarray(b2, np.float32).reshape(1, C2), (128, 1)),
        "ident": np.eye(128, dtype=np.float32),
        "drow1": drow1,
        "drow2": drow2,
    }
    in_maps = []
    for c in range(N_CORES):
        xs = np.zeros((F_IN, SP), np.float32)
        xs[:, 0:S] = x[c * S:(c + 1) * S].T
        m = {"xT": xs, **common}
        cd = cores_data[c]
        for k in range(NCH):
            m[f"idxe{k}"] = cd["idx_e"][k]
            m[f"idxer{k}"] = cd["idx_er"][k]
            m[f"idxuz{k}"] = cd["idx_uz"][k]
        in_maps.append(m)

    res = run_bass_kernel_spmd(nc, in_maps, core_ids=list(range(N_CORES)),
                               trace=PROFILE)
    LAST_RESULTS = res.results
    LAST_EXEC_NS = res.exec_time_ns
    out = np.zeros((V, C2), np.float32)
    for c in range(N_CORES):
        out[c * S:(c + 1) * S] = res.results[c]["out_shard"][0:S]
    return out


def _to_bf16(a):
    import ml_dtypes
    return a.astype(ml_dtypes.bfloat16)


# revision 38
# speedup vs baseline: 1.0048x; 1.0048x over previous
"""GAT 2-layer node classifier on 8 Trainium2 NeuronCores.

Strategy (1D dst-node partitioning + chunked src gather):
  - dst nodes sharded contiguously across 8 cores (12500 each, natural order)
  - node phase computes per-node rows [feat64|el8|er8|pad] fp16 (256B) which
    are AllGathered into a 100360-row table (12545 rows per core incl. one
    dummy row whose el is -3e4 so padded slots contribute exp() = 0).
  - the per-edge gather uses dma_gather (Ant Q7 kernel, int16 indices):
    the table is addressed through 4 windows of 2 source cores each
    (25090 rows < 32767).  Per window ("chunk") each core re-sorts its dst
    nodes by within-chunk degree and packs 128 dst per group with in-edge
    slots padded to the group's max chunk-degree, giving a near-tight grid
    (~2.4% padding).  Per chunk the partial softmax sums [U64|Z8] are
    written to a DRAM table in chunk order; a renorm phase gathers the 4
    partials back into natural order (gather-based realignment), divides,
    applies bias/relu and runs the layer-2 projection.  Layer 2 repeats the
    edge phase with the same grids/indices on the layer-2 table.
"""

import sys
import types

import numpy as np

# ---------------------------------------------------------------------------
# environment shims (self-contained: only touches in-process state)
# ---------------------------------------------------------------------------


def _ensure_axon_hooks():
    """concourse.bass_utils imports antenv.axon_hooks when tracing under
    axon; some images lack the module. Provide an in-process shim and
    register the real ctypes NTFF hook so trace=True yields exec times."""
    try:
        import antenv.axon_hooks  # noqa: F401
        return
    except Exception:
        pass
    try:
        import antenv
    except Exception:
        return
    mod = types.ModuleType("antenv.axon_hooks")
    mod._hook = None

    def set_axon_ntff_profile_hook(hook):
        mod._hook = hook

    def get_axon_ntff_profile_hook():
        return mod._hook

    mod.set_axon_ntff_profile_hook = set_axon_ntff_profile_hook
    mod.get_axon_ntff_profile_hook = get_axon_ntff_profile_hook
    sys.modules["antenv.axon_hooks"] = mod
    antenv.axon_hooks = mod
    try:
        from trn_agent_boot.trn_boot import _ntff_profile_via_ctypes
        hook = _ntff_profile_via_ctypes("/opt/axon/libaxon_pjrt.so")
        if hook is not None:
            mod._hook = hook
    except Exception:
        pass


_ensure_axon_hooks()

import concourse.bass as bass          # noqa: E402
import concourse.mybir as mybir        # noqa: E402
import concourse.tile as tile          # noqa: E402
from concourse import library_config   # noqa: E402
from concourse.vector_clock import ScopedClock  # noqa: E402
from concourse.bass_utils import run_bass_kernel_spmd  # noqa: E402

F32 = mybir.dt.float32
F16 = mybir.dt.float16
BF16 = mybir.dt.bfloat16
I16 = mybir.dt.int16
AF = mybir.ActivationFunctionType
OP = mybir.AluOpType
AX = mybir.AxisListType


def _patched_drain_and_barrier(self, tick_clock, wait_clock):
    # this walrus build rejects multi-wait instructions; emit one wait per
    # nop before the tail drain instead of stacking them on the drain.
    nc = self.nc
    probe = nc.sync.nop(nofuse=True)
    wait_clock.add_sem_waits(probe.ins, ScopedClock({None: tick_clock.global_clock}))
    waits = list(probe.ins.sync_info.on_wait or []) if probe.ins.sync_info else []
    if waits:
        probe.ins.sync_info = mybir.SyncInfo(on_wait=[waits[0]], on_update=[])
        for w in waits[1:]:
            nop = nc.sync.nop(nofuse=True)
            nop.ins.sync_info = mybir.SyncInfo(on_wait=[w], on_update=[])
    nc.sync.drain()
    nc.all_engine_barrier()
    popped = nc._tile_sem_poison_stack.pop()
    assert popped is self._sem_poison
    nc.clear_and_free_semaphores(list(self.sems.allocated().values()))
    nc.all_engine_barrier()


tile.TileContext._drain_and_barrier = _patched_drain_and_barrier


def _split_waits(nc, max_waits=1):
    """Post-pass: any instruction carrying more than max_waits sem-waits gets
    preceding same-engine NoOps carrying the excess."""
    uid = [0]
    for f in nc.m.functions:
        for bb in f.blocks:
            new_insts = []
            for inst in bb.instructions:
                si = getattr(inst, "sync_info", None)
                if si is not None and si.on_wait and len(si.on_wait) > max_waits:
                    waits = list(si.on_wait)
                    excess, keep = waits[:-max_waits], waits[-max_waits:]
                    for i in range(0, len(excess), max_waits):
                        uid[0] += 1
                        new_insts.append(mybir.InstNoOp(
                            name=f"waitsplit-{uid[0]}-{inst.name}",
                            sync_info=mybir.SyncInfo(
                                on_wait=excess[i:i + max_waits], on_update=[]),
                            bass_nofuse=True,
                            engine=inst.engine,
                        ))
                    inst.sync_info = mybir.SyncInfo(
                        on_wait=keep, on_update=list(si.on_update or []))
                new_insts.append(inst)
            bb.instructions = new_insts


# ---------------------------------------------------------------------------
# problem constants (hardcoded per spec)
# ---------------------------------------------------------------------------
N_CORES = 8
V = 100000            # nodes
S = V // N_CORES      # nodes per core shard (12500)
F_IN = 256
H1, D1 = 8, 8         # layer-1 heads x dim
HD1 = H1 * D1         # 64
C2 = 40               # classes (layer-2 single head)
NEG_SLOPE = 0.2
EL_NEG = -30000.0     # dummy-row attention logit (exp -> 0)
G = (S + 127) // 128  # 98 groups of 128 dst nodes
SP = G * 128          # 12544 padded shard size
W = SP + 1            # 12545 table rows per core (last = dummy)
NCH = 4               # src chunks (2 cores per window)
WIN = 2 * W           # 25090 rows per chunk window
ROWB = 128            # fp16 elements per table row (256B)
CB = 48               # max grid columns per gather batch (SBUF window)
SC = 8                # grid columns per dma_gather sub-call (<=1024 idxs)
RB = 8                # renorm groups per realign gather call (<=1024 idxs)
MAXL = 32             # per-(group,chunk) slot-column bound

# module-level knobs (test harness pokes these)
PROFILE = False
DEBUG = False
PHASES = 6   # debug: 1=node 2=+AG1 3=+edgeL1 4=+renorm1 5=+AG2+edgeL2 6=full
EDGE_SUB = 4  # debug: 1=ER only 2=+F gathers 3=+scores 4=full
SKIP_AG = False  # debug: skip collectives
LAST_EXEC_NS = None
LAST_RESULTS = None


# ---------------------------------------------------------------------------
# host-side graph preprocessing (integer work only)
# ---------------------------------------------------------------------------

def _wrap16(flat):
    """int array [n] (n % 16 == 0) -> dma_gather idx layout [128, n//16]:
    idx i at partition i%16, col i//16, replicated to 128 partitions."""
    a = np.asarray(flat, np.int16).reshape(-1, 16).T    # [16, n//16]
    return np.tile(a, (8, 1)).copy()                    # [128, n//16]


def _host_prep(src, dst):
    src = np.asarray(src).astype(np.int64)
    dst = np.asarray(dst).astype(np.int64)
    order = np.argsort(dst, kind="stable")
    src_s = src[order]
    dst_s = dst[order]
    bounds = np.searchsorted(dst_s, np.arange(N_CORES + 1) * S)

    # table row of a node: core cc = v // S, row = cc*W + (v - cc*S)
    cc_all = src_s // S
    trow_all = cc_all * W + (src_s - cc_all * S)        # global table row
    chunk_all = cc_all // 2                             # 0..3
    lrow_all = trow_all - chunk_all * WIN               # window-local row

    per_core = []
    Lkg_shared = np.zeros((NCH, G), np.int64)
    for c in range(N_CORES):
        sl = slice(bounds[c], bounds[c + 1])
        ld = dst_s[sl] - c * S
        ck = chunk_all[sl]
        lr = lrow_all[sl]
        chunks = []
        for k in range(NCH):
            m = ck == k
            ldk, lrk = ld[m], lr[m]
            degk = np.bincount(ldk, minlength=SP).astype(np.int64)  # ext ids
            pk = np.argsort(-degk, kind="stable").astype(np.int64)
            inv = np.empty(SP, np.int64)
            inv[pk] = np.arange(SP)
            Lg = degk[pk[np.arange(G) * 128]]
            Lkg_shared[k] = np.maximum(Lkg_shared[k], Lg)
            # edge list sorted by (chunk-rank of dst, stable)
            o2 = np.argsort(inv[ldk], kind="stable")
            chunks.append(dict(degk=degk, pk=pk, inv=inv,
                               e_lrow=lrk[o2], e_pos=inv[ldk][o2]))
        per_core.append(chunks)

    # shared grid: per chunk, columns for groups with L>0
    grids = []          # per chunk: list of (g, L, colstart)
    cols_k = []
    for k in range(NCH):
        col = 0
        gl = []
        for g in range(G):
            L = int(Lkg_shared[k][g])
            if L == 0:
                continue
            gl.append((g, L, col))
            col += L
        grids.append(gl)
        cols_k.append(col)

    # per-core slot/realign index arrays
    cores_data = []
    for c in range(N_CORES):
        idx_e, idx_er, idx_uz = [], [], []
        for k in range(NCH):
            ch = per_core[c][k]
            dummy = 2 * (k * W) + W - 1 - k * WIN       # first core's dummy row
            dummy = W - 1                                # window-local: core 2k dummy
            ncols = cols_k[k]
            slots = np.full((ncols, 128), dummy, np.int64)
            degk, pk = ch["degk"], ch["pk"]
            starts = np.zeros(SP, np.int64)
            np.cumsum(degk[pk][:-1], out=starts[1:])     # start in e_lrow per rank
            for (g, L, col) in grids[k]:
                ranks = np.arange(g * 128, (g + 1) * 128)
                dg = degk[pk[ranks]]
                st = starts[ranks]
                ar = np.arange(L)
                mask = ar[None, :] < dg[:, None]
                pos = np.minimum(st[:, None] + ar[None, :],
                                 max(len(ch["e_lrow"]) - 1, 0))
                vals = (ch["e_lrow"][pos] if len(ch["e_lrow"])
                        else np.zeros_like(pos))
                gs = np.where(mask, vals, dummy)         # [128, L]
                slots[col:col + L, :] = gs.T
            assert slots.max() < WIN
            idx_e.append(_wrap16(slots.reshape(-1)))
            # er realign: grid position i -> local shard-table row pk[i]
            idx_er.append(_wrap16(ch["pk"]))
            # uz realign: natural position i -> chunk rank inv[i]
            idx_uz.append(_wrap16(ch["inv"]))
        cores_data.append(dict(idx_e=idx_e, idx_er=idx_er, idx_uz=idx_uz))

    # gather call batches per chunk: runs of groups with sum(L) <= CB
    batches = []        # per chunk: list of (colstart, ncols, [(g,L,off)...])
    for k in range(NCH):
        bl = []
        cur = []
        cur_cols = 0
        cur_start = 0
        for (g, L, col) in grids[k]:
            if cur_cols + L > CB:
                bl.append((cur_start, cur_cols, cur))
                cur, cur_cols, cur_start = [], 0, col
            cur.append((g, L, cur_cols))
            cur_cols += L
        if cur:
            bl.append((cur_start, cur_cols, cur))
        batches.append(bl)

    return cores_data, grids, cols_k, batches, Lkg_shared


# ---------------------------------------------------------------------------
# device program
# ---------------------------------------------------------------------------

def _build_program(cols_k, batches, grids):
    nc = bass.Bass("TRN2", target_bir_lowering=False, debug=False,
                   num_devices=N_CORES, num_swdge_queues=4)

    def dram_in(name, shape, dt=F32):
        return nc.dram_tensor(name, list(shape), dt, kind="ExternalInput").ap()

    xT = dram_in("xT", [F_IN, SP])
    W1e_d = dram_in("W1e", [F_IN, 80])
    W2e_d = dram_in("W2e", [HD1, 42])
    b1_d = dram_in("b1", [128, HD1])
    b2_d = dram_in("b2", [128, C2])
    ident_d = dram_in("ident", [128, 128])
    drow1_d = dram_in("drow1", [1, ROWB], F16)
    drow2_d = dram_in("drow2", [1, ROWB], F16)
    idxe_d = [dram_in(f"idxe{k}", [128, cols_k[k] * 8], I16) for k in range(NCH)]
    idxer_d = [dram_in(f"idxer{k}", [128, SP // 16], I16) for k in range(NCH)]
    idxuz_d = [dram_in(f"idxuz{k}", [128, SP // 16], I16) for k in range(NCH)]

    out_shard = nc.dram_tensor("out_shard", [SP, C2], F32,
                               kind="ExternalOutput").ap()

    with tile.TileContext(nc) as tc:
        with (
            tc.tile_pool(name="dram", bufs=1, space="DRAM") as dram,
            tc.tile_pool(name="const", bufs=1) as constp,
            tc.tile_pool(name="node", bufs=2) as nodep,
            tc.tile_pool(name="idxp", bufs=2) as idxp,
            tc.tile_pool(name="erp", bufs=1) as erp,
            tc.tile_pool(name="uzp", bufs=1) as uzp,
            tc.tile_pool(name="gath", bufs=3) as gath,
            tc.tile_pool(name="work", bufs=2) as work,
            tc.tile_pool(name="rz", bufs=2) as rzp,
            tc.tile_pool(name="psum", bufs=2, space="PSUM") as psum,
        ):
            # Pool has only 48 allocatable registers; dma_gather burns one
            # per to_reg(num_idxs) call — cache by value.
            _regs = {}
            def nreg(v):
                if v not in _regs:
                    _regs[v] = nc.gpsimd.to_reg(v)
                return _regs[v]

            _q = [0]
            def qrr():
                _q[0] = (_q[0] + 1) % 4
                return _q[0]

            # ---- persistent DRAM tables -----------------------------------
            t1_shard = dram.tile([W, ROWB], F16)
            t1_full = dram.tile([N_CORES * W, ROWB], F16)
            t2_shard = dram.tile([W, ROWB], F16)
            t2_full = dram.tile([N_CORES * W, ROWB], F16)
            UZ1 = [dram.tile([SP, ROWB], F16, name=f"UZ1_{k}", tag=f"UZ1_{k}")
                   for k in range(NCH)]
            UZ2 = [dram.tile([SP, ROWB], F16, name=f"UZ2_{k}", tag=f"UZ2_{k}")
                   for k in range(NCH)]
            t1_full[:].tensor.mls.addr_space = "Shared"
            t2_full[:].tensor.mls.addr_space = "Shared"

            # ---- constants into SBUF --------------------------------------
            _cn = [0]
            def const_load(src_ap, shape, dt=F32):
                _cn[0] += 1
                t = constp.tile(shape, dt, tag=f"const{_cn[0]}")
                nc.sync.dma_start(out=t[:], in_=src_ap)
                return t

            W1a = const_load(W1e_d[0:128, :], [128, 80])
            W1b = const_load(W1e_d[128:256, :], [128, 80])
            W2sb = const_load(W2e_d[:, :], [HD1, 42])
            b1 = const_load(b1_d[:, :], [128, HD1])
            b2 = const_load(b2_d[:, :], [128, C2])
            ident = const_load(ident_d[:, :], [128, 128])

            # dummy rows of the gather tables
            nc.sync.dma_start(out=t1_shard[W - 1:W, :], in_=drow1_d[:, :])
            nc.sync.dma_start(out=t2_shard[W - 1:W, :], in_=drow2_d[:, :])

            # ---- node phase: [feat64|el8|er8] fp16 rows for own shard -----
            NB = 4
            for n in range(G):
                if n % NB == 0:
                    nw = min(NB, G - n) * 128
                    cs4 = slice(n * 128, n * 128 + nw)
                    xa = nodep.tile([128, NB * 128], F32, tag="xa")
                    xb = nodep.tile([128, NB * 128], F32, tag="xb")
                    nc.scalar.dma_start(out=xa[:, 0:nw], in_=xT[0:128, cs4])
                    nc.scalar.dma_start(out=xb[:, 0:nw], in_=xT[128:256, cs4])
                k = (n % NB) * 128
                p1 = psum.tile([128, 80], F32, tag="p1")
                nc.tensor.matmul(out=p1[:], lhsT=xa[:, k:k + 128], rhs=W1a[:],
                                 start=True, stop=False)
                nc.tensor.matmul(out=p1[:], lhsT=xb[:, k:k + 128], rhs=W1b[:],
                                 start=False, stop=True)
                if n % NB == 0:
                    S4 = nodep.tile([128, NB * ROWB], F16, tag="S4")
                j = n % NB
                nc.scalar.copy(out=S4[:, j * ROWB:j * ROWB + 80], in_=p1[:])
                if n % NB == NB - 1 or n == G - 1:
                    m = n % NB + 1
                    a = (n - m + 1) * 128
                    nc.sync.dma_start(
                        out=t1_shard[a:a + m * 128, :]
                            .rearrange("(j p) r -> p j r", p=128),
                        in_=S4[:, 0:m * ROWB]
                            .rearrange("p (j r) -> p j r", r=ROWB))

            # ---- AllGather layer-1 table ----------------------------------
            if PHASES >= 2 and not SKIP_AG:
                nc.gpsimd.collective_compute(
                    "AllGather", OP.bypass,
                    replica_groups=[list(range(N_CORES))],
                    ins=[t1_shard[0:W, :].opt()],
                    outs=[t1_full[:, :].opt()],
                )

            # ---- edge phase (shared between both layers) ------------------
            def edge_phase(tfull, tshard, UZ, FC, elc, erc):
                """FC: feature count (64/40); elc/erc: el/er col in row."""
                ctx = nc.allow_low_precision(
                    reason="per-chunk partials in fp16; <=24-term sums")
                ctx.__enter__()
                for k in range(NCH):
                    win = tfull[k * WIN:(k + 1) * WIN, :]
                    idxs = idxp.tile([128, cols_k[k] * 8], I16, tag="idxs")
                    nc.sync.dma_start(out=idxs[:], in_=idxe_d[k][:, :])
                    ER = erp.tile([128, G * ROWB], F16, tag="ER")
                    if EDGE_SUB == 0:
                        nc.vector.memset(ER[:], 0.0)
                    else:
                        EB = 8   # groups per call (1024-idx gather limit)
                        for g0 in range(0, G, EB):
                            ng = min(EB, G - g0)
                            nc.gpsimd.dma_gather(
                                ER[:, g0 * ROWB:(g0 + ng) * ROWB]
                                    .rearrange("p (b r) -> p b r", r=ROWB),
                                tshard[0:W, :],
                                ier[k][:, g0 * 8:(g0 + ng) * 8],
                                ng * 128, nreg(ng * 128), ROWB,
                                queue_num=qrr())
                    ERv = ER[:].rearrange("p (b r) -> p b r", r=ROWB)
                    UZSB = uzp.tile([128, G * ROWB], F16, tag="UZSB")
                    if EDGE_SUB < 4:
                        nc.vector.memset(UZSB[:], 0.0)
                    if EDGE_SUB <= 1:
                        nc.sync.dma_start(
                            out=UZ[k][0:SP, :]
                                .rearrange("(g p) r -> p g r", p=128),
                            in_=UZSB[:].rearrange("p (g r) -> p g r", r=ROWB))
                        continue
                    for (colstart, ncols, gl) in batches[k]:
                        F = gath.tile([128, CB * ROWB], F16, tag="F")
                        for c0 in range(0, ncols, SC):
                            nc2 = min(SC, ncols - c0)
                            nc.gpsimd.dma_gather(
                                F[:, c0 * ROWB:(c0 + nc2) * ROWB]
                                    .rearrange("p (b r) -> p b r", r=ROWB),
                                win,
                                idxs[:, (colstart + c0) * 8:
                                     (colstart + c0 + nc2) * 8],
                                nc2 * 128, nreg(nc2 * 128), ROWB,
                                queue_num=qrr())
                        if EDGE_SUB == 2:
                            continue
                        Fv = F[:, 0:ncols * ROWB].rearrange(
                            "p (b r) -> p b r", r=ROWB)
                        nh = NH[FC]
                        A = work.tile([128, CB * 8], F32, tag="A")
                        for (g, L, off) in gl:
                            nc.vector.tensor_add(
                                out=A[:, off * nh:(off + L) * nh].rearrange(
                                    "p (l h) -> p l h", h=nh),
                                in0=Fv[:, off:off + L, elc:elc + nh],
                                in1=ERv[:, g:g + 1, erc:erc + nh]
                                    .to_broadcast([128, L, nh]))
                        na = ncols * nh
                        LR = work.tile([128, CB * 8], F32, tag="LR")
                        nc.vector.tensor_scalar_mul(
                            out=LR[:, 0:na], in0=A[:, 0:na], scalar1=NEG_SLOPE)
                        nc.vector.tensor_tensor(
                            out=LR[:, 0:na], in0=A[:, 0:na], in1=LR[:, 0:na],
                            op=OP.max)
                        EX = work.tile([128, CB * 8], F16, tag="EX")
                        nc.scalar.activation(
                            out=EX[:, 0:na], in_=LR[:, 0:na], func=AF.Exp)
                        if EDGE_SUB == 3:
                            continue
                        for (g, L, off) in gl:
                            nc.vector.reduce_sum(
                                out=UZSB[:, g * ROWB + FC:g * ROWB + FC + nh],
                                in_=EX[:, off * nh:(off + L) * nh].rearrange(
                                    "p (l h) -> p h l", h=nh), axis=AX.X)
                            P = work.tile([128, MAXL * HD1], F16, tag="P")
                            nc.vector.tensor_mul(
                                out=P[:, 0:L * FC].rearrange(
                                    "p (l h j) -> p l h j", h=nh, j=FC // nh),
                                in0=Fv[:, off:off + L, 0:FC].rearrange(
                                    "p l (h j) -> p l h j", h=nh),
                                in1=EX[:, off * nh:(off + L) * nh].rearrange(
                                    "p (l h) -> p l h", h=nh)
                                    .rearrange("p l (h o) -> p l h o", o=1)
                                    .to_broadcast([128, L, nh, FC // nh]))
                            nc.vector.reduce_sum(
                                out=UZSB[:, g * ROWB:g * ROWB + FC],
                                in_=P[:, 0:L * FC].rearrange(
                                    "p (l f) -> p f l", f=FC), axis=AX.X)
                    # groups with no columns in this chunk: zero their U/Z
                    have = {g for (g, L, off) in
                            [t for (_, _, gl2) in batches[k] for t in gl2]}
                    for g in range(G):
                        if g not in have:
                            nc.vector.memset(
                                UZSB[:, g * ROWB:g * ROWB + FC + NH[FC]], 0.0)
                    nc.sync.dma_start(
                        out=UZ[k][0:SP, :].rearrange("(g p) r -> p g r", p=128),
                        in_=UZSB[:].rearrange("p (g r) -> p g r", r=ROWB))
                ctx.__exit__(None, None, None)

            NH = {HD1: H1, C2: 1}

            # ---- renorm + next-layer node phase ---------------------------
            def renorm_phase(UZ, idxuz, FC, emit_group):
                nh = NH[FC]
                nb = (G + RB - 1) // RB
                for b in range(nb):
                    g0 = b * RB
                    ng = min(RB, G - g0)
                    RZ = []
                    for k in range(NCH):
                        r = rzp.tile([128, RB * ROWB], F16, tag=f"RZ{k}")
                        nc.gpsimd.dma_gather(
                            r[:, 0:ng * ROWB]
                                .rearrange("p (b r) -> p b r", r=ROWB),
                            UZ[k][0:SP, :],
                            iuz[k][:, g0 * 8:(g0 + ng) * 8],
                            ng * 128, nreg(ng * 128), ROWB,
                            queue_num=qrr())
                        RZ.append(r)
                    nb2 = ng * ROWB
                    T0 = work.tile([128, RB * ROWB], F32, tag="T0")
                    nc.vector.tensor_add(out=T0[:, 0:nb2],
                                         in0=RZ[0][:, 0:nb2], in1=RZ[1][:, 0:nb2])
                    T1 = work.tile([128, RB * ROWB], F32, tag="T1")
                    nc.vector.tensor_add(out=T1[:, 0:nb2],
                                         in0=RZ[2][:, 0:nb2], in1=RZ[3][:, 0:nb2])
                    UZf = work.tile([128, RB * ROWB], F32, tag="UZf")
                    nc.vector.tensor_add(out=UZf[:, 0:nb2],
                                         in0=T0[:, 0:nb2], in1=T1[:, 0:nb2])
                    for gg in range(ng):
                        g = g0 + gg
                        c0 = gg * ROWB
                        rinv = work.tile([128, 8], F32, tag="rinv")
                        nc.vector.reciprocal(out=rinv[:, 0:nh],
                                             in_=UZf[:, c0 + FC:c0 + FC + nh])
                        O = work.tile([128, HD1], F32, tag="O")
                        nc.vector.tensor_mul(
                            out=O[:, 0:FC].rearrange("p (h j) -> p h j", h=nh),
                            in0=UZf[:, c0:c0 + FC].rearrange(
                                "p (h j) -> p h j", h=nh),
                            in1=rinv[:, 0:nh].rearrange("p (h o) -> p h o", o=1)
                                .to_broadcast([128, nh, FC // nh]))
                        emit_group(g, O)

            # layer-1 renorm group: h = relu(O + b1); project to layer-2 row
            S42 = [None]
            def emit_l1(g, O):
                Ht = work.tile([128, HD1], F32, tag="Ht")
                nc.vector.tensor_add(out=Ht[:], in0=O[:, 0:HD1], in1=b1[:])
                nc.scalar.activation(out=Ht[:], in_=Ht[:], func=AF.Relu)
                pT = psum.tile([HD1, 128], F32, tag="pT")
                nc.tensor.transpose(out=pT[:], in_=Ht[:], identity=ident[:])
                hT = work.tile([HD1, 128], F32, tag="hT")
                nc.scalar.copy(out=hT[:], in_=pT[:])
                p2 = psum.tile([128, 42], F32, tag="p2")
                nc.tensor.matmul(out=p2[:], lhsT=hT[:], rhs=W2sb[:],
                                 start=True, stop=True)
                if g % NB == 0:
                    S42[0] = nodep.tile([128, NB * ROWB], F16, name="S42",
                                        tag="S42")
                j = g % NB
                nc.scalar.copy(out=S42[0][:, j * ROWB:j * ROWB + 42], in_=p2[:])
                if g % NB == NB - 1 or g == G - 1:
                    m = g % NB + 1
                    a = (g - m + 1) * 128
                    nc.sync.dma_start(
                        out=t2_shard[a:a + m * 128, :]
                            .rearrange("(j p) r -> p j r", p=128),
                        in_=S42[0][:, 0:m * ROWB]
                            .rearrange("p (j r) -> p j r", r=ROWB))

            # layer-2 renorm group: out = O + b2
            O4 = [None]
            def emit_l2(g, O):
                if g % NB == 0:
                    O4[0] = nodep.tile([128, NB * C2], F32, name="O4", tag="O4")
                j = g % NB
                nc.vector.tensor_add(out=O4[0][:, j * C2:(j + 1) * C2],
                                     in0=O[:, 0:C2], in1=b2[:])
                if g % NB == NB - 1 or g == G - 1:
                    m = g % NB + 1
                    a = (g - m + 1) * 128
                    nc.sync.dma_start(
                        out=out_shard[a:a + m * 128, :]
                            .rearrange("(j p) r -> p j r", p=128),
                        in_=O4[0][:, 0:m * C2]
                            .rearrange("p (j r) -> p j r", r=C2))

            # ---- idx tiles for er/uz realign (persistent, small) ----------
            ier = [const_load(idxer_d[k][:, :], [128, SP // 16], I16)
                   for k in range(NCH)]
            iuz = [const_load(idxuz_d[k][:, :], [128, SP // 16], I16)
                   for k in range(NCH)]

            # ---- run the phases -------------------------------------------
            if PHASES >= 3:
                edge_phase(t1_full, t1_shard, UZ1, HD1, 64, 72)
            if PHASES >= 4:
                renorm_phase(UZ1, iuz, HD1, emit_l1)
            if PHASES >= 5:
                nc.gpsimd.collective_compute(
                    "AllGather", OP.bypass,
                    replica_groups=[list(range(N_CORES))],
                    ins=[t2_shard[0:W, :].opt()],
                    outs=[t2_full[:, :].opt()],
                )
                edge_phase(t2_full, t2_shard, UZ2, C2, 40, 41)
            if PHASES >= 6:
                renorm_phase(UZ2, iuz, C2, emit_l2)
            if PHASES < 6:
                zo = nodep.tile([128, C2], F32, tag="zo", name="zo")
                nc.vector.memset(zo[:], 0.0)
                nc.sync.dma_start(
                    out=out_shard[0:128, :], in_=zo[:])

    # Raw Bass skips Bacc's library/ISA lowering passes; without them the
    # NEFF compiler sees empty .instr on extended insts -> "ISA wrong length".
    import bass_rust as _bass_rust
    inst_type_to_lib_mask = {}
    for lib in library_config.all_libraries:
        for t in lib.instructions:
            inst_type_to_lib_mask[t] = (inst_type_to_lib_mask.get(t, 0)
                                        | (1 << lib.index))
    _bass_rust.insert_library_loads(
        nc, inst_type_to_lib_mask, len(library_config.all_libraries),
        library_config.standard.index)
    mybir.codegen_inst_isa_subclasses(nc)

    _split_waits(nc)
    return nc


# ---------------------------------------------------------------------------
# entry point
# ---------------------------------------------------------------------------

def kernel(x, W1, attn_l1, attn_r1, b1, W2, attn_l2, attn_r2, b2, src, dst):
    global LAST_EXEC_NS, LAST_RESULTS
    x = np.asarray(x, np.float32)
    cores_data, grids, cols_k, batches, Lkg = _host_prep(src, dst)
    nc = _build_program(cols_k, batches, grids)

    W1f = np.asarray(W1, np.float32)
    al1 = np.asarray(attn_l1, np.float32).reshape(H1, D1)
    ar1 = np.asarray(attn_r1, np.float32).reshape(H1, D1)
    Wl = (W1f.reshape(F_IN, H1, D1) * al1[None]).sum(-1)
    Wr = (W1f.reshape(F_IN, H1, D1) * ar1[None]).sum(-1)
    W1e = np.concatenate([W1f, Wl, Wr], axis=1).astype(np.float32)

    W2f = np.asarray(W2, np.float32)
    al2 = np.asarray(attn_l2, np.float32).reshape(1, C2)
    ar2 = np.asarray(attn_r2, np.float32).reshape(1, C2)
    Wl2 = (W2f * al2).sum(-1, keepdims=True)
    Wr2 = (W2f * ar2).sum(-1, keepdims=True)
    W2e = np.concatenate([W2f, Wl2, Wr2], axis=1).astype(np.float32)

    drow1 = np.zeros((1, ROWB), np.float16)
    drow1[0, 64:72] = EL_NEG
    drow2 = np.zeros((1, ROWB), np.float16)
    drow2[0, 40] = EL_NEG

    common = {
        "W1e": W1e,
        "W2e": W2e,
        "b1": np.tile(np.asarray(b1, np.float32).reshape(1, HD1), (128, 1)),
        "b2": np.tile(np.asarray(b2, np.float32).reshape(1, C2), (128, 1)),
        "ident": np.eye(128, dtype=np.float32),
        "drow1": drow1,
        "drow2": drow2,
    }
    in_maps = []
    for c in range(N_CORES):
        xs = np.zeros((F_IN, SP), np.float32)
        xs[:, 0:S] = x[c * S:(c + 1) * S].T
        m = {"xT": xs, **common}
        cd = cores_data[c]
        for k in range(NCH):
            m[f"idxe{k}"] = cd["idx_e"][k]
            m[f"idxer{k}"] = cd["idx_er"][k]
            m[f"idxuz{k}"] = cd["idx_uz"][k]
        in_maps.append(m)

    res = run_bass_kernel_spmd(nc, in_maps, core_ids=list(range(N_CORES)),
                               trace=PROFILE)
    LAST_RESULTS = res.results
    LAST_EXEC_NS = res.exec_time_ns
    out = np.zeros((V, C2), np.float32)
    for c in range(N_CORES):
        out[c * S:(c + 1) * S] = res.results[c]["out_shard"][0:S]
    return out


def _to_bf16(a):
    import ml_dtypes
    return a.astype(ml_dtypes.bfloat16)


# revision 39
# speedup vs baseline: 1.0055x; 1.0007x over previous
"""GAT 2-layer node classifier on 8 Trainium2 NeuronCores.

Strategy (1D dst-node partitioning + chunked src gather):
  - dst nodes sharded contiguously across 8 cores (12500 each, natural order)
  - node phase computes per-node rows [feat64|el8|er8|pad] fp16 (256B) which
    are AllGathered into a 100360-row table (12545 rows per core incl. one
    dummy row whose el is -3e4 so padded slots contribute exp() = 0).
  - the per-edge gather uses dma_gather (Ant Q7 kernel, int16 indices):
    the table is addressed through 4 windows of 2 source cores each
    (25090 rows < 32767).  Per window ("chunk") each core re-sorts its dst
    nodes by within-chunk degree and packs 128 dst per group with in-edge
    slots padded to the group's max chunk-degree, giving a near-tight grid
    (~2.4% padding).  Per chunk the partial softmax sums [U64|Z8] are
    written to a DRAM table in chunk order; a renorm phase gathers the 4
    partials back into natural order (gather-based realignment), divides,
    applies bias/relu and runs the layer-2 projection.  Layer 2 repeats the
    edge phase with the same grids/indices on the layer-2 table.
"""

import sys
import types

import numpy as np

# ---------------------------------------------------------------------------
# environment shims (self-contained: only touches in-process state)
# ---------------------------------------------------------------------------


def _ensure_axon_hooks():
    """concourse.bass_utils imports antenv.axon_hooks when tracing under
    axon; some images lack the module. Provide an in-process shim and
    register the real ctypes NTFF hook so trace=True yields exec times."""
    try:
        import antenv.axon_hooks  # noqa: F401
        return
    except Exception:
        pass
    try:
        import antenv
    except Exception:
        return
    mod = types.ModuleType("antenv.axon_hooks")
    mod._hook = None

    def set_axon_ntff_profile_hook(hook):
        mod._hook = hook

    def get_axon_ntff_profile_hook():
        return mod._hook

    mod.set_axon_ntff_profile_hook = set_axon_ntff_profile_hook
    mod.get_axon_ntff_profile_hook = get_axon_ntff_profile_hook
    sys.modules["antenv.axon_hooks"] = mod
    antenv.axon_hooks = mod
    try:
        from trn_agent_boot.trn_boot import _ntff_profile_via_ctypes
        hook = _ntff_profile_via_ctypes("/opt/axon/libaxon_pjrt.so")
        if hook is not None:
            mod._hook = hook
    except Exception:
        pass


_ensure_axon_hooks()

import concourse.bass as bass          # noqa: E402
import concourse.mybir as mybir        # noqa: E402
import concourse.tile as tile          # noqa: E402
from concourse import library_config   # noqa: E402
from concourse.vector_clock import ScopedClock  # noqa: E402
from concourse.bass_utils import run_bass_kernel_spmd  # noqa: E402

F32 = mybir.dt.float32
F16 = mybir.dt.float16
BF16 = mybir.dt.bfloat16
I16 = mybir.dt.int16
AF = mybir.ActivationFunctionType
OP = mybir.AluOpType
AX = mybir.AxisListType


def _patched_drain_and_barrier(self, tick_clock, wait_clock):
    # this walrus build rejects multi-wait instructions; emit one wait per
    # nop before the tail drain instead of stacking them on the drain.
    nc = self.nc
    probe = nc.sync.nop(nofuse=True)
    wait_clock.add_sem_waits(probe.ins, ScopedClock({None: tick_clock.global_clock}))
    waits = list(probe.ins.sync_info.on_wait or []) if probe.ins.sync_info else []
    if waits:
        probe.ins.sync_info = mybir.SyncInfo(on_wait=[waits[0]], on_update=[])
        for w in waits[1:]:
            nop = nc.sync.nop(nofuse=True)
            nop.ins.sync_info = mybir.SyncInfo(on_wait=[w], on_update=[])
    nc.sync.drain()
    nc.all_engine_barrier()
    popped = nc._tile_sem_poison_stack.pop()
    assert popped is self._sem_poison
    nc.clear_and_free_semaphores(list(self.sems.allocated().values()))
    nc.all_engine_barrier()


tile.TileContext._drain_and_barrier = _patched_drain_and_barrier


def _split_waits(nc, max_waits=1):
    """Post-pass: any instruction carrying more than max_waits sem-waits gets
    preceding same-engine NoOps carrying the excess."""
    uid = [0]
    for f in nc.m.functions:
        for bb in f.blocks:
            new_insts = []
            for inst in bb.instructions:
                si = getattr(inst, "sync_info", None)
                if si is not None and si.on_wait and len(si.on_wait) > max_waits:
                    waits = list(si.on_wait)
                    excess, keep = waits[:-max_waits], waits[-max_waits:]
                    for i in range(0, len(excess), max_waits):
                        uid[0] += 1
                        new_insts.append(mybir.InstNoOp(
                            name=f"waitsplit-{uid[0]}-{inst.name}",
                            sync_info=mybir.SyncInfo(
                                on_wait=excess[i:i + max_waits], on_update=[]),
                            bass_nofuse=True,
                            engine=inst.engine,
                        ))
                    inst.sync_info = mybir.SyncInfo(
                        on_wait=keep, on_update=list(si.on_update or []))
                new_insts.append(inst)
            bb.instructions = new_insts


# ---------------------------------------------------------------------------
# problem constants (hardcoded per spec)
# ---------------------------------------------------------------------------
N_CORES = 8
V = 100000            # nodes
S = V // N_CORES      # nodes per core shard (12500)
F_IN = 256
H1, D1 = 8, 8         # layer-1 heads x dim
HD1 = H1 * D1         # 64
C2 = 40               # classes (layer-2 single head)
NEG_SLOPE = 0.2
EL_NEG = -30000.0     # dummy-row attention logit (exp -> 0)
G = (S + 127) // 128  # 98 groups of 128 dst nodes
SP = G * 128          # 12544 padded shard size
W = SP + 1            # 12545 table rows per core (last = dummy)
NCH = 4               # src chunks (2 cores per window)
WIN = 2 * W           # 25090 rows per chunk window
ROWB = 128            # fp16 elements per table row (256B)
CB = 48               # max grid columns per gather batch (SBUF window)
SC = 8                # grid columns per dma_gather sub-call (<=1024 idxs)
RB = 8                # renorm groups per realign gather call (<=1024 idxs)
MAXL = 32             # per-(group,chunk) slot-column bound

# module-level knobs (test harness pokes these)
PROFILE = False
DEBUG = False
PHASES = 6   # debug: 1=node 2=+AG1 3=+edgeL1 4=+renorm1 5=+AG2+edgeL2 6=full
EDGE_SUB = 4  # debug: 1=ER only 2=+F gathers 3=+scores 4=full
SKIP_AG = False  # debug: skip collectives
LAST_EXEC_NS = None
LAST_RESULTS = None


# ---------------------------------------------------------------------------
# host-side graph preprocessing (integer work only)
# ---------------------------------------------------------------------------

def _wrap16(flat):
    """int array [n] (n % 16 == 0) -> dma_gather idx layout [128, n//16]:
    idx i at partition i%16, col i//16, replicated to 128 partitions."""
    a = np.asarray(flat, np.int16).reshape(-1, 16).T    # [16, n//16]
    return np.tile(a, (8, 1)).copy()                    # [128, n//16]


def _host_prep(src, dst):
    src = np.asarray(src).astype(np.int64)
    dst = np.asarray(dst).astype(np.int64)
    order = np.argsort(dst, kind="stable")
    src_s = src[order]
    dst_s = dst[order]
    bounds = np.searchsorted(dst_s, np.arange(N_CORES + 1) * S)

    # table row of a node: core cc = v // S, row = cc*W + (v - cc*S)
    cc_all = src_s // S
    trow_all = cc_all * W + (src_s - cc_all * S)        # global table row
    chunk_all = cc_all // 2                             # 0..3
    lrow_all = trow_all - chunk_all * WIN               # window-local row

    per_core = []
    Lkg_shared = np.zeros((NCH, G), np.int64)
    for c in range(N_CORES):
        sl = slice(bounds[c], bounds[c + 1])
        ld = dst_s[sl] - c * S
        ck = chunk_all[sl]
        lr = lrow_all[sl]
        chunks = []
        for k in range(NCH):
            m = ck == k
            ldk, lrk = ld[m], lr[m]
            degk = np.bincount(ldk, minlength=SP).astype(np.int64)  # ext ids
            pk = np.argsort(-degk, kind="stable").astype(np.int64)
            inv = np.empty(SP, np.int64)
            inv[pk] = np.arange(SP)
            Lg = degk[pk[np.arange(G) * 128]]
            Lkg_shared[k] = np.maximum(Lkg_shared[k], Lg)
            # edge list sorted by (chunk-rank of dst, stable)
            o2 = np.argsort(inv[ldk], kind="stable")
            chunks.append(dict(degk=degk, pk=pk, inv=inv,
                               e_lrow=lrk[o2], e_pos=inv[ldk][o2]))
        per_core.append(chunks)

    # shared grid: per chunk, columns for groups with L>0
    grids = []          # per chunk: list of (g, L, colstart)
    cols_k = []
    for k in range(NCH):
        col = 0
        gl = []
        for g in range(G):
            L = int(Lkg_shared[k][g])
            if L == 0:
                continue
            gl.append((g, L, col))
            col += L
        grids.append(gl)
        cols_k.append(col)

    # per-core slot/realign index arrays
    cores_data = []
    for c in range(N_CORES):
        idx_e, idx_er, idx_uz = [], [], []
        for k in range(NCH):
            ch = per_core[c][k]
            dummy = 2 * (k * W) + W - 1 - k * WIN       # first core's dummy row
            dummy = W - 1                                # window-local: core 2k dummy
            ncols = cols_k[k]
            slots = np.full((ncols, 128), dummy, np.int64)
            degk, pk = ch["degk"], ch["pk"]
            starts = np.zeros(SP, np.int64)
            np.cumsum(degk[pk][:-1], out=starts[1:])     # start in e_lrow per rank
            for (g, L, col) in grids[k]:
                ranks = np.arange(g * 128, (g + 1) * 128)
                dg = degk[pk[ranks]]
                st = starts[ranks]
                ar = np.arange(L)
                mask = ar[None, :] < dg[:, None]
                pos = np.minimum(st[:, None] + ar[None, :],
                                 max(len(ch["e_lrow"]) - 1, 0))
                vals = (ch["e_lrow"][pos] if len(ch["e_lrow"])
                        else np.zeros_like(pos))
                gs = np.where(mask, vals, dummy)         # [128, L]
                slots[col:col + L, :] = gs.T
            assert slots.max() < WIN
            idx_e.append(_wrap16(slots.reshape(-1)))
            # er realign: grid position i -> local shard-table row pk[i]
            idx_er.append(_wrap16(ch["pk"]))
            # uz realign: natural position i -> chunk rank inv[i]
            idx_uz.append(_wrap16(ch["inv"]))
        cores_data.append(dict(idx_e=idx_e, idx_er=idx_er, idx_uz=idx_uz))

    # gather call batches per chunk: runs of groups with sum(L) <= CB
    batches = []        # per chunk: list of (colstart, ncols, [(g,L,off)...])
    for k in range(NCH):
        bl = []
        cur = []
        cur_cols = 0
        cur_start = 0
        for (g, L, col) in grids[k]:
            if cur_cols + L > CB:
                bl.append((cur_start, cur_cols, cur))
                cur, cur_cols, cur_start = [], 0, col
            cur.append((g, L, cur_cols))
            cur_cols += L
        if cur:
            bl.append((cur_start, cur_cols, cur))
        batches.append(bl)

    return cores_data, grids, cols_k, batches, Lkg_shared


# ---------------------------------------------------------------------------
# device program
# ---------------------------------------------------------------------------

def _build_program(cols_k, batches, grids):
    nc = bass.Bass("TRN2", target_bir_lowering=False, debug=False,
                   num_devices=N_CORES, num_swdge_queues=4)

    def dram_in(name, shape, dt=F32):
        return nc.dram_tensor(name, list(shape), dt, kind="ExternalInput").ap()

    xT = dram_in("xT", [F_IN, SP])
    W1e_d = dram_in("W1e", [F_IN, 80])
    W2e_d = dram_in("W2e", [HD1, 42])
    b1_d = dram_in("b1", [128, HD1])
    b2_d = dram_in("b2", [128, C2])
    ident_d = dram_in("ident", [128, 128])
    drow1_d = dram_in("drow1", [1, ROWB], F16)
    drow2_d = dram_in("drow2", [1, ROWB], F16)
    idxe_d = [dram_in(f"idxe{k}", [128, cols_k[k] * 8], I16) for k in range(NCH)]
    idxer_d = [dram_in(f"idxer{k}", [128, SP // 16], I16) for k in range(NCH)]
    idxuz_d = [dram_in(f"idxuz{k}", [128, SP // 16], I16) for k in range(NCH)]

    out_shard = nc.dram_tensor("out_shard", [SP, C2], F32,
                               kind="ExternalOutput").ap()

    with tile.TileContext(nc) as tc:
        with (
            tc.tile_pool(name="dram", bufs=1, space="DRAM") as dram,
            tc.tile_pool(name="const", bufs=1) as constp,
            tc.tile_pool(name="node", bufs=2) as nodep,
            tc.tile_pool(name="idxp", bufs=2) as idxp,
            tc.tile_pool(name="erp", bufs=1) as erp,
            tc.tile_pool(name="uzp", bufs=1) as uzp,
            tc.tile_pool(name="gath", bufs=3) as gath,
            tc.tile_pool(name="work", bufs=2) as work,
            tc.tile_pool(name="rz", bufs=2) as rzp,
            tc.tile_pool(name="psum", bufs=2, space="PSUM") as psum,
        ):
            # Pool has only 48 allocatable registers; dma_gather burns one
            # per to_reg(num_idxs) call — cache by value.
            _regs = {}
            def nreg(v):
                if v not in _regs:
                    _regs[v] = nc.gpsimd.to_reg(v)
                return _regs[v]

            _q = [0]
            def qrr():
                _q[0] = (_q[0] + 1) % 4
                return _q[0]

            # ---- persistent DRAM tables -----------------------------------
            t1_shard = dram.tile([W, ROWB], F16)
            t1_full = dram.tile([N_CORES * W, ROWB], F16)
            t2_shard = dram.tile([W, ROWB], F16)
            t2_full = dram.tile([N_CORES * W, ROWB], F16)
            UZ1 = [dram.tile([SP, ROWB], F16, name=f"UZ1_{k}", tag=f"UZ1_{k}")
                   for k in range(NCH)]
            UZ2 = [dram.tile([SP, ROWB], F16, name=f"UZ2_{k}", tag=f"UZ2_{k}")
                   for k in range(NCH)]
            t1_full[:].tensor.mls.addr_space = "Shared"
            t2_full[:].tensor.mls.addr_space = "Shared"

            # ---- constants into SBUF --------------------------------------
            _cn = [0]
            def const_load(src_ap, shape, dt=F32):
                _cn[0] += 1
                t = constp.tile(shape, dt, tag=f"const{_cn[0]}")
                nc.sync.dma_start(out=t[:], in_=src_ap)
                return t

            W1a = const_load(W1e_d[0:128, :], [128, 80])
            W1b = const_load(W1e_d[128:256, :], [128, 80])
            W2sb = const_load(W2e_d[:, :], [HD1, 42])
            b1 = const_load(b1_d[:, :], [128, HD1])
            b2 = const_load(b2_d[:, :], [128, C2])
            ident = const_load(ident_d[:, :], [128, 128])

            # dummy rows of the gather tables
            nc.sync.dma_start(out=t1_shard[W - 1:W, :], in_=drow1_d[:, :])
            nc.sync.dma_start(out=t2_shard[W - 1:W, :], in_=drow2_d[:, :])

            # ---- node phase: [feat64|el8|er8] fp16 rows for own shard -----
            NB = 4
            for n in range(G):
                if n % NB == 0:
                    nw = min(NB, G - n) * 128
                    cs4 = slice(n * 128, n * 128 + nw)
                    xa = nodep.tile([128, NB * 128], F32, tag="xa")
                    xb = nodep.tile([128, NB * 128], F32, tag="xb")
                    nc.scalar.dma_start(out=xa[:, 0:nw], in_=xT[0:128, cs4])
                    nc.scalar.dma_start(out=xb[:, 0:nw], in_=xT[128:256, cs4])
                k = (n % NB) * 128
                p1 = psum.tile([128, 80], F32, tag="p1")
                nc.tensor.matmul(out=p1[:], lhsT=xa[:, k:k + 128], rhs=W1a[:],
                                 start=True, stop=False)
                nc.tensor.matmul(out=p1[:], lhsT=xb[:, k:k + 128], rhs=W1b[:],
                                 start=False, stop=True)
                if n % NB == 0:
                    S4 = nodep.tile([128, NB * ROWB], F16, tag="S4")
                j = n % NB
                nc.scalar.copy(out=S4[:, j * ROWB:j * ROWB + 80], in_=p1[:])
                if n % NB == NB - 1 or n == G - 1:
                    m = n % NB + 1
                    a = (n - m + 1) * 128
                    nc.sync.dma_start(
                        out=t1_shard[a:a + m * 128, :]
                            .rearrange("(j p) r -> p j r", p=128),
                        in_=S4[:, 0:m * ROWB]
                            .rearrange("p (j r) -> p j r", r=ROWB))

            # ---- AllGather layer-1 table ----------------------------------
            if PHASES >= 2 and not SKIP_AG:
                nc.gpsimd.collective_compute(
                    "AllGather", OP.bypass,
                    replica_groups=[list(range(N_CORES))],
                    ins=[t1_shard[0:W, :].opt()],
                    outs=[t1_full[:, :].opt()],
                )

            # ---- edge phase (shared between both layers) ------------------
            def edge_phase(tfull, tshard, UZ, FC, elc, erc):
                """FC: feature count (64/40); elc/erc: el/er col in row."""
                ctx = nc.allow_low_precision(
                    reason="per-chunk partials in fp16; <=24-term sums")
                ctx.__enter__()
                for k in range(NCH):
                    win = tfull[k * WIN:(k + 1) * WIN, :]
                    idxs = idxp.tile([128, cols_k[k] * 8], I16, tag="idxs")
                    nc.sync.dma_start(out=idxs[:], in_=idxe_d[k][:, :])
                    ER = erp.tile([128, G * ROWB], F16, tag="ER")
                    if EDGE_SUB == 0:
                        nc.vector.memset(ER[:], 0.0)
                    else:
                        EB = 8   # groups per call (1024-idx gather limit)
                        for g0 in range(0, G, EB):
                            ng = min(EB, G - g0)
                            nc.gpsimd.dma_gather(
                                ER[:, g0 * ROWB:(g0 + ng) * ROWB]
                                    .rearrange("p (b r) -> p b r", r=ROWB),
                                tshard[0:W, :],
                                ier[k][:, g0 * 8:(g0 + ng) * 8],
                                ng * 128, nreg(ng * 128), ROWB,
                                queue_num=qrr())
                    ERv = ER[:].rearrange("p (b r) -> p b r", r=ROWB)
                    UZSB = uzp.tile([128, G * ROWB], F16, tag="UZSB")
                    if EDGE_SUB < 4:
                        nc.vector.memset(UZSB[:], 0.0)
                    if EDGE_SUB <= 1:
                        nc.sync.dma_start(
                            out=UZ[k][0:SP, :]
                                .rearrange("(g p) r -> p g r", p=128),
                            in_=UZSB[:].rearrange("p (g r) -> p g r", r=ROWB))
                        continue
                    for (colstart, ncols, gl) in batches[k]:
                        F = gath.tile([128, CB * ROWB], F16, tag="F")
                        for c0 in range(0, ncols, SC):
                            nc2 = min(SC, ncols - c0)
                            nc.gpsimd.dma_gather(
                                F[:, c0 * ROWB:(c0 + nc2) * ROWB]
                                    .rearrange("p (b r) -> p b r", r=ROWB),
                                win,
                                idxs[:, (colstart + c0) * 8:
                                     (colstart + c0 + nc2) * 8],
                                nc2 * 128, nreg(nc2 * 128), ROWB,
                                queue_num=qrr())
                        if EDGE_SUB == 2:
                            continue
                        Fv = F[:, 0:ncols * ROWB].rearrange(
                            "p (b r) -> p b r", r=ROWB)
                        nh = NH[FC]
                        A = work.tile([128, CB * 8], F32, tag="A")
                        for (g, L, off) in gl:
                            nc.vector.tensor_add(
                                out=A[:, off * nh:(off + L) * nh].rearrange(
                                    "p (l h) -> p l h", h=nh),
                                in0=Fv[:, off:off + L, elc:elc + nh],
                                in1=ERv[:, g:g + 1, erc:erc + nh]
                                    .to_broadcast([128, L, nh]))
                        na = ncols * nh
                        LR = work.tile([128, CB * 8], F32, tag="LR")
                        nc.vector.tensor_scalar_mul(
                            out=LR[:, 0:na], in0=A[:, 0:na], scalar1=NEG_SLOPE)
                        nc.vector.tensor_tensor(
                            out=LR[:, 0:na], in0=A[:, 0:na], in1=LR[:, 0:na],
                            op=OP.max)
                        EX = work.tile([128, CB * 8], F16, tag="EX")
                        nc.scalar.activation(
                            out=EX[:, 0:na], in_=LR[:, 0:na], func=AF.Exp)
                        if EDGE_SUB == 3:
                            continue
                        for (g, L, off) in gl:
                            nc.vector.reduce_sum(
                                out=UZSB[:, g * ROWB + FC:g * ROWB + FC + nh],
                                in_=EX[:, off * nh:(off + L) * nh].rearrange(
                                    "p (l h) -> p h l", h=nh), axis=AX.X)
                            P = work.tile([128, MAXL * HD1], F16, tag="P")
                            nc.vector.tensor_mul(
                                out=P[:, 0:L * FC].rearrange(
                                    "p (l h j) -> p l h j", h=nh, j=FC // nh),
                                in0=Fv[:, off:off + L, 0:FC].rearrange(
                                    "p l (h j) -> p l h j", h=nh),
                                in1=EX[:, off * nh:(off + L) * nh].rearrange(
                                    "p (l h) -> p l h", h=nh)
                                    .rearrange("p l (h o) -> p l h o", o=1)
                                    .to_broadcast([128, L, nh, FC // nh]))
                            nc.vector.reduce_sum(
                                out=UZSB[:, g * ROWB:g * ROWB + FC],
                                in_=P[:, 0:L * FC].rearrange(
                                    "p (l f) -> p f l", f=FC), axis=AX.X)
                    # groups with no columns in this chunk: zero their U/Z
                    have = {g for (g, L, off) in
                            [t for (_, _, gl2) in batches[k] for t in gl2]}
                    for g in range(G):
                        if g not in have:
                            nc.vector.memset(
                                UZSB[:, g * ROWB:g * ROWB + FC + NH[FC]], 0.0)
                    nc.sync.dma_start(
                        out=UZ[k][0:SP, :].rearrange("(g p) r -> p g r", p=128),
                        in_=UZSB[:].rearrange("p (g r) -> p g r", r=ROWB))
                ctx.__exit__(None, None, None)

            NH = {HD1: H1, C2: 1}

            # ---- renorm + next-layer node phase ---------------------------
            def renorm_phase(UZ, idxuz, FC, emit_group):
                nh = NH[FC]
                nb = (G + RB - 1) // RB
                for b in range(nb):
                    g0 = b * RB
                    ng = min(RB, G - g0)
                    RZ = []
                    for k in range(NCH):
                        r = rzp.tile([128, RB * ROWB], F16, tag=f"RZ{k}")
                        nc.gpsimd.dma_gather(
                            r[:, 0:ng * ROWB]
                                .rearrange("p (b r) -> p b r", r=ROWB),
                            UZ[k][0:SP, :],
                            iuz[k][:, g0 * 8:(g0 + ng) * 8],
                            ng * 128, nreg(ng * 128), ROWB,
                            queue_num=qrr())
                        RZ.append(r)
                    nb2 = ng * ROWB
                    T0 = work.tile([128, RB * ROWB], F32, tag="T0")
                    nc.vector.tensor_add(out=T0[:, 0:nb2],
                                         in0=RZ[0][:, 0:nb2], in1=RZ[1][:, 0:nb2])
                    T1 = work.tile([128, RB * ROWB], F32, tag="T1")
                    nc.vector.tensor_add(out=T1[:, 0:nb2],
                                         in0=RZ[2][:, 0:nb2], in1=RZ[3][:, 0:nb2])
                    UZf = work.tile([128, RB * ROWB], F32, tag="UZf")
                    nc.vector.tensor_add(out=UZf[:, 0:nb2],
                                         in0=T0[:, 0:nb2], in1=T1[:, 0:nb2])
                    for gg in range(ng):
                        g = g0 + gg
                        c0 = gg * ROWB
                        rinv = work.tile([128, 8], F32, tag="rinv")
                        nc.vector.reciprocal(out=rinv[:, 0:nh],
                                             in_=UZf[:, c0 + FC:c0 + FC + nh])
                        O = work.tile([128, HD1], F32, tag="O")
                        nc.vector.tensor_mul(
                            out=O[:, 0:FC].rearrange("p (h j) -> p h j", h=nh),
                            in0=UZf[:, c0:c0 + FC].rearrange(
                                "p (h j) -> p h j", h=nh),
                            in1=rinv[:, 0:nh].rearrange("p (h o) -> p h o", o=1)
                                .to_broadcast([128, nh, FC // nh]))
                        emit_group(g, O)

            # layer-1 renorm group: h = relu(O + b1); project to layer-2 row
            S42 = [None]
            def emit_l1(g, O):
                Ht = work.tile([128, HD1], F32, tag="Ht")
                nc.vector.tensor_add(out=Ht[:], in0=O[:, 0:HD1], in1=b1[:])
                nc.scalar.activation(out=Ht[:], in_=Ht[:], func=AF.Relu)
                pT = psum.tile([HD1, 128], F32, tag="pT")
                nc.tensor.transpose(out=pT[:], in_=Ht[:], identity=ident[:])
                hT = work.tile([HD1, 128], F32, tag="hT")
                nc.scalar.copy(out=hT[:], in_=pT[:])
                p2 = psum.tile([128, 42], F32, tag="p2")
                nc.tensor.matmul(out=p2[:], lhsT=hT[:], rhs=W2sb[:],
                                 start=True, stop=True)
                if g % NB == 0:
                    S42[0] = nodep.tile([128, NB * ROWB], F16, name="S42",
                                        tag="S42")
                j = g % NB
                nc.scalar.copy(out=S42[0][:, j * ROWB:j * ROWB + 42], in_=p2[:])
                if g % NB == NB - 1 or g == G - 1:
                    m = g % NB + 1
                    a = (g - m + 1) * 128
                    nc.sync.dma_start(
                        out=t2_shard[a:a + m * 128, :]
                            .rearrange("(j p) r -> p j r", p=128),
                        in_=S42[0][:, 0:m * ROWB]
                            .rearrange("p (j r) -> p j r", r=ROWB))

            # layer-2 renorm group: out = O + b2
            O4 = [None]
            def emit_l2(g, O):
                if g % NB == 0:
                    O4[0] = nodep.tile([128, NB * C2], F32, name="O4", tag="O4")
                j = g % NB
                nc.vector.tensor_add(out=O4[0][:, j * C2:(j + 1) * C2],
                                     in0=O[:, 0:C2], in1=b2[:])
                if g % NB == NB - 1 or g == G - 1:
                    m = g % NB + 1
                    a = (g - m + 1) * 128
                    nc.sync.dma_start(
                        out=out_shard[a:a + m * 128, :]
                            .rearrange("(j p) r -> p j r", p=128),
                        in_=O4[0][:, 0:m * C2]
                            .rearrange("p (j r) -> p j r", r=C2))

            # ---- idx tiles for er/uz realign (persistent, small) ----------
            ier = [const_load(idxer_d[k][:, :], [128, SP // 16], I16)
                   for k in range(NCH)]
            iuz = [const_load(idxuz_d[k][:, :], [128, SP // 16], I16)
                   for k in range(NCH)]

            # ---- run the phases -------------------------------------------
            if PHASES >= 3:
                edge_phase(t1_full, t1_shard, UZ1, HD1, 64, 72)
            if PHASES >= 4:
                renorm_phase(UZ1, iuz, HD1, emit_l1)
            if PHASES >= 5:
                nc.gpsimd.collective_compute(
                    "AllGather", OP.bypass,
                    replica_groups=[list(range(N_CORES))],
                    ins=[t2_shard[0:W, :].opt()],
                    outs=[t2_full[:, :].opt()],
                )
                edge_phase(t2_full, t2_shard, UZ2, C2, 40, 41)
            if PHASES >= 6:
                renorm_phase(UZ2, iuz, C2, emit_l2)
            if PHASES < 6:
                zo = nodep.tile([128, C2], F32, tag="zo", name="zo")
                nc.vector.memset(zo[:], 0.0)
                nc.sync.dma_start(
                    out=out_shard[0:128, :], in_=zo[:])

    # Raw Bass skips Bacc's library/ISA lowering passes; without them the
    # NEFF compiler sees empty .instr on extended insts -> "ISA wrong length".
    import bass_rust as _bass_rust
    inst_type_to_lib_mask = {}
    for lib in library_config.all_libraries:
        for t in lib.instructions:
            inst_type_to_lib_mask[t] = (inst_type_to_lib_mask.get(t, 0)
                                        | (1 << lib.index))
    _bass_rust.insert_library_loads(
        nc, inst_type_to_lib_mask, len(library_config.all_libraries),
        library_config.standard.index)
    mybir.codegen_inst_isa_subclasses(nc)

    _split_waits(nc)
    return nc


# ---------------------------------------------------------------------------
# entry point
# ---------------------------------------------------------------------------

def kernel(x, W1, attn_l1, attn_r1, b1, W2, attn_l2, attn_r2, b2, src, dst):
    global LAST_EXEC_NS, LAST_RESULTS
    x = np.asarray(x, np.float32)
    cores_data, grids, cols_k, batches, Lkg = _host_prep(src, dst)
    nc = _build_program(cols_k, batches, grids)

    W1f = np.asarray(W1, np.float32)
    al1 = np.asarray(attn_l1, np.float32).reshape(H1, D1)
    ar1 = np.asarray(attn_r1, np.float32).reshape(H1, D1)
    Wl = (W1f.reshape(F_IN, H1, D1) * al1[None]).sum(-1)
    Wr = (W1f.reshape(F_IN, H1, D1) * ar1[None]).sum(-1)
    W1e = np.concatenate([W1f, Wl, Wr], axis=1).astype(np.float32)

    W2f = np.asarray(W2, np.float32)
    al2 = np.asarray(attn_l2, np.float32).reshape(1, C2)
    ar2 = np.asarray(attn_r2, np.float32).reshape(1, C2)
    Wl2 = (W2f * al2).sum(-1, keepdims=True)
    Wr2 = (W2f * ar2).sum(-1, keepdims=True)
    W2e = np.concatenate([W2f, Wl2, Wr2], axis=1).astype(np.float32)

    drow1 = np.zeros((1, ROWB), np.float16)
    drow1[0, 64:72] = EL_NEG
    drow2 = np.zeros((1, ROWB), np.float16)
    drow2[0, 40] = EL_NEG

    common = {
        "W1e": W1e,
        "W2e": W2e,
        "b1": np.tile(np.asarray(b1, np.float32).reshape(1, HD1), (128, 1)),
        "b2": np.tile(np.asarray(b2, np.float32).reshape(1, C2), (128, 1)),
        "ident": np.eye(128, dtype=np.float32),
        "drow1": drow1,
        "drow2": drow2,
    }
    in_maps = []
    for c in range(N_CORES):
        xs = np.zeros((F_IN, SP), np.float32)
        xs[:, 0:S] = x[c * S:(c + 1) * S].T
        m = {"xT": xs, **common}
        cd = cores_data[c]
        for k in range(NCH):
            m[f"idxe{k}"] = cd["idx_e"][k]
            m[f"idxer{k}"] = cd["idx_er"][k]
            m[f"idxuz{k}"] = cd["idx_uz"][k]
        in_maps.append(m)

    res = run_bass_kernel_spmd(nc, in_maps, core_ids=list(range(N_CORES)),
                               trace=PROFILE)
    LAST_RESULTS = res.results
    LAST_EXEC_NS = res.exec_time_ns
    out = np.zeros((V, C2), np.float32)
    for c in range(N_CORES):
        out[c * S:(c + 1) * S] = res.results[c]["out_shard"][0:S]
    return out

